# revision 1
# baseline (speedup 1.0000x reference)
"""v5: v3 with diag-half-first phase ordering (drains DMA backlog).

Per-core local column order is [own 2048 rows | other 2048 rows], so the
diagonal (symmetric) block of G is always local columns [0, 2048) — SPMD-
uniform across cores. Within that block, strictly-lower [128,512] tiles are
not recomputed: their adjacency masks are PE-transposed copies of the
strictly-upper tiles' masks (bf16 0/1, retained in SBUF). That removes
216 of 1152 matmuls per core at the cost of 96 cheap transpose ops.
Host un-permutes the output columns (a half-swap for odd cores) during
assembly at zero extra copy cost.
"""

import sys

for _p in ("/opt/trn_rl_repo", "/root/.axon_site/_ro/trn_rl_repo"):
    if _p not in sys.path:
        sys.path.append(_p)

import numpy as np

B, C, N = 4, 384, 4096
HALF = N // 2
KT = C // 128
NCORES = 2 * B
PPF_09 = 1.2815515655446004
EPS = 1e-12
SCALE = 256.0
RB = HALF // 128       # 16 row blocks per core
JH = 2
JT = 4
HEADC = 512            # head-start chunk (first matmul tile's moving cols)

_compiled_nc = None


def _build_nc():
    import concourse.bacc as bacc
    import concourse.tile as tile
    import concourse.mybir as mybir

    f32 = mybir.dt.float32
    f16 = mybir.dt.float16
    bf16 = mybir.dt.bfloat16
    i32 = mybir.dt.int32
    Alu = mybir.AluOpType
    Act = mybir.ActivationFunctionType

    nc = bacc.Bacc("TRN2", target_bir_lowering=False, debug=False)

    xh0_d = nc.dram_tensor("xh0", [128, 2, HEADC], f16, kind="ExternalInput")
    x_d = nc.dram_tensor("xhl", [KT, 128, 2, N], f16, kind="ExternalInput")
    thr_d = nc.dram_tensor("thr", [128, 1], f32, kind="ExternalInput")
    row_d = nc.dram_tensor("rowp1", [128, RB], f32, kind="ExternalInput")
    col_d = nc.dram_tensor("colp1", [128, N], f32, kind="ExternalInput")
    idn_d = nc.dram_tensor("ident", [128, 128], bf16, kind="ExternalInput")
    e0_d = nc.dram_tensor("e0", [HALF, N], i32, kind="ExternalOutput")
    e1_d = nc.dram_tensor("e1", [HALF, N], i32, kind="ExternalOutput")

    def tclass(rb, jt):
        if rb <= 4 * jt - 1:
            return "upper"
        if rb >= 4 * jt + 4:
            return "lower"
        return "cross"

    with tile.TileContext(nc) as tc:
        with tc.tile_pool(name="const", bufs=1) as cpool, \
             tc.tile_pool(name="psum", bufs=5, space="PSUM") as psum, \
             tc.tile_pool(name="pstp", bufs=2, space="PSUM") as pstp, \
             tc.tile_pool(name="e1ip", bufs=3) as e1ip, \
             tc.tile_pool(name="e1if", bufs=2) as e1if, \
             tc.tile_pool(name="e0ip", bufs=2) as e0ip, \
             tc.tile_pool(name="e0if", bufs=2) as e0if, \
             tc.tile_pool(name="outp", bufs=3) as outp, \
             tc.tile_pool(name="outf", bufs=2) as outf:
            xh0 = cpool.tile([128, 2, HEADC], f16, name="xh0")
            nc.sync.dma_start(out=xh0[:], in_=xh0_d.ap())
            xts = [cpool.tile([128, 2, N], f16, name=f"x{k}") for k in range(KT)]
            for k in range(KT):
                nc.sync.dma_start(out=xts[k][:], in_=x_d[k])
            thr_t = cpool.tile([128, 1], f32, name="thr_t")
            nc.sync.dma_start(out=thr_t[:], in_=thr_d.ap())
            row_t = cpool.tile([128, RB], f32, name="row_t")
            nc.sync.dma_start(out=row_t[:], in_=row_d.ap())
            col_t = cpool.tile([128, N], f32, name="col_t")
            nc.sync.dma_start(out=col_t[:], in_=col_d.ap())
            idn_t = cpool.tile([128, 128], bf16, name="idn_t")
            nc.sync.dma_start(out=idn_t[:], in_=idn_d.ap())
            masks = {}
            for jt in range(JT):
                for rb in range(RB):
                    if tclass(rb, jt) == "upper":
                        masks[(rb, jt)] = cpool.tile(
                            [128, 512], bf16, name=f"mk_{rb}_{jt}")

            def mm_group(ps, i0, j0, first_tile):
                m = 0
                for k in range(KT):
                    src = xh0 if (first_tile and k == 0) else xts[k]
                    hi = src[:, 0, :]
                    lo = src[:, 1, :]
                    for a, bb in ((hi, hi), (hi, lo), (lo, hi)):
                        nc.tensor.matmul(
                            ps[:],
                            a[:, i0:i0 + 128],
                            bb[:, j0:j0 + 512],
                            start=(m == 0), stop=(m == 3 * KT - 1),
                        )
                        m += 1

            def post(e1i_ap, rb, jcol, width, e0i_pool, out_pool):
                e0i = e0i_pool.tile([128, width], f32, name="e0i")
                nc.vector.tensor_scalar(
                    e0i[:], e1i_ap, 0.0, row_t[:, rb:rb + 1],
                    op0=Alu.is_gt, op1=Alu.mult,
                )
                e0o = out_pool.tile([128, width], i32, name="e0o")
                e1o = out_pool.tile([128, width], i32, name="e1o")
                nc.scalar.activation(e0o[:], e0i[:], Act.Copy, bias=-1.0)
                nc.scalar.activation(e1o[:], e1i_ap, Act.Copy, bias=-1.0)
                i0 = rb * 128
                nc.sync.dma_start(
                    out=e0_d.ap()[i0:i0 + 128, jcol:jcol + width], in_=e0o[:])
                nc.sync.dma_start(
                    out=e1_d.ap()[i0:i0 + 128, jcol:jcol + width], in_=e1o[:])

            # Phase 1: all diagonal (symmetric) halves. The mirror tiles
            # make output production cheap/bursty here, which the DMA absorbs
            # while it is otherwise idle after the input phase.
            for rb in range(RB):
                i0 = rb * 128
                e1i = e1ip.tile([128, HALF], f32, name="e1i")
                for jt in range(JT):
                    j0 = jt * 512
                    cls = tclass(rb, jt)
                    sl = e1i[:, j0:j0 + 512]
                    if cls == "lower":
                        pst = pstp.tile([128, 512], bf16, name="pst")
                        for q in range(4):
                            src = masks[(4 * jt + q, rb // 4)]
                            nc.tensor.transpose(
                                pst[:, q * 128:(q + 1) * 128],
                                src[:, (rb % 4) * 128:(rb % 4) * 128 + 128],
                                idn_t[:],
                            )
                        nc.vector.tensor_tensor(
                            sl, pst[:], col_t[:, j0:j0 + 512], op=Alu.mult)
                    else:
                        ps = psum.tile([128, 512], f32, name="ps")
                        mm_group(ps, i0, j0, rb == 0 and jt == 0)
                        if cls == "upper":
                            mk = masks[(rb, jt)]
                            nc.vector.tensor_scalar(
                                mk[:], ps[:], thr_t[:], None, op0=Alu.is_gt)
                            nc.vector.tensor_tensor(
                                sl, mk[:], col_t[:, j0:j0 + 512], op=Alu.mult)
                        else:
                            nc.vector.scalar_tensor_tensor(
                                sl, ps[:], thr_t[:], col_t[:, j0:j0 + 512],
                                op0=Alu.is_gt, op1=Alu.mult)
                post(e1i[:], rb, 0, HALF, e0ip, outp)

            # Phase 2: all off-diagonal halves (pure matmul; production rate
            # below DMA bandwidth, so the queue drains before the end).
            for rb in range(RB):
                i0 = rb * 128
                last_block = (rb == RB - 1)
                if not last_block:
                    e1b = e1ip.tile([128, HALF], f32, name="e1i")
                    for jt in range(JT):
                        j0 = HALF + jt * 512
                        ps = psum.tile([128, 512], f32, name="ps")
                        mm_group(ps, i0, j0, False)
                        nc.vector.scalar_tensor_tensor(
                            e1b[:, jt * 512:(jt + 1) * 512], ps[:], thr_t[:],
                            col_t[:, j0:j0 + 512], op0=Alu.is_gt, op1=Alu.mult)
                    post(e1b[:], rb, HALF, HALF, e0ip, outp)
                else:
                    for jt in range(JT):
                        j0 = HALF + jt * 512
                        ps = psum.tile([128, 512], f32, name="ps")
                        mm_group(ps, i0, j0, False)
                        e1s = e1if.tile([128, 512], f32, name="e1s")
                        nc.vector.scalar_tensor_tensor(
                            e1s[:], ps[:], thr_t[:],
                            col_t[:, j0:j0 + 512], op0=Alu.is_gt, op1=Alu.mult)
                        post(e1s[:], rb, j0, 512, e0if, outf)
    nc.compile()
    return nc


def get_nc():
    global _compiled_nc
    if _compiled_nc is None:
        _compiled_nc = _build_nc()
    return _compiled_nc


def make_inputs(x):
    import ml_dtypes

    xs = np.asarray(x)[:, :, :, 0]                      # (B, C, N) fp32
    nrm = np.sqrt(np.sum(xs * xs, axis=1, keepdims=True))
    xn = xs / np.maximum(nrm, EPS)

    ident = np.eye(128, dtype=ml_dtypes.bfloat16)
    Nsq = float(N) * float(N)
    in_maps = []
    for b in range(B):
        xb64 = xn[b].astype(np.float64)
        s = xb64.sum(axis=1)
        M = xb64 @ xb64.T
        sum_g = float(s @ s)
        sum_g2 = float((M * M).sum())
        mean = (2.0 * sum_g - 2.0 * Nsq) / Nsq
        s2 = 4.0 * sum_g2 - 8.0 * sum_g + 4.0 * Nsq
        var = (s2 - Nsq * mean * mean) / (Nsq - 1.0)
        t_b = (mean + PPF_09 * np.sqrt(var) + 2.0) / 2.0
        thr_dev = np.full((128, 1), t_b * SCALE * SCALE, np.float32)

        xbs = (xn[b] * SCALE).astype(np.float32)
        for h in range(2):
            # local column order: own rows first, then the other half
            xloc = np.concatenate(
                [xbs[:, h * HALF:(h + 1) * HALF],
                 xbs[:, (1 - h) * HALF:(2 - h) * HALF]], axis=1)  # (C, N)
            hi = xloc.astype(np.float16)
            lo = (xloc - hi.astype(np.float32)).astype(np.float16)
            xhl = np.stack([hi.reshape(KT, 128, N),
                            lo.reshape(KT, 128, N)], axis=2)
            xh0 = np.ascontiguousarray(xhl[0, :, :, :HEADC])
            gcols = np.concatenate(
                [h * HALF + np.arange(HALF), (1 - h) * HALF + np.arange(HALF)])
            colp1 = np.ascontiguousarray(np.broadcast_to(
                (b * N + gcols + 1).astype(np.float32), (128, N)))
            rows = (b * N + h * HALF
                    + (np.arange(RB)[None, :] * 128 + np.arange(128)[:, None])
                    + 1).astype(np.float32)
            in_maps.append({
                "xh0": xh0,
                "xhl": np.ascontiguousarray(xhl),
                "thr": thr_dev,
                "rowp1": np.ascontiguousarray(rows),
                "colp1": colp1,
                "ident": ident,
            })
    return in_maps


def assemble(results):
    out = np.empty((2, B * N * N), np.int32)
    for c in range(NCORES):
        b, h = divmod(c, 2)
        base = b * N * N + h * HALF * N
        for plane, key in ((0, "e0"), (1, "e1")):
            dst = out[plane, base:base + HALF * N].reshape(HALF, 2, HALF)
            src = results[c][key].reshape(HALF, 2, HALF)
            if h == 0:
                dst[:] = src
            else:
                dst[:, 0, :] = src[:, 1, :]
                dst[:, 1, :] = src[:, 0, :]
    return out


def kernel(x):
    from concourse.bass_utils import run_bass_kernel_spmd

    nc = get_nc()
    in_maps = make_inputs(x)
    res = run_bass_kernel_spmd(nc, in_maps, list(range(NCORES)))
    return assemble(res.results)



# revision 2
# speedup vs baseline: 2.8149x; 2.8149x over previous
"""v6: fp32r single-pass Gram + global upper-triangle only + uint8 mask output.

Per batch, G = xn^T xn is exactly symmetric, so only the upper-triangular
[128,512] tiles are computed on device (host mirrors the mask). The two
cores of a batch split the upper triangle evenly and SPMD-uniformly:
within every 512-column chunk, core h owns the two 128-row blocks
{2h, 2h+1}; the host permutes each chunk's four 128-blocks per core
([0,1,2,3] / [2,3,0,1]) so both cores run the identical program
(stationary = local blocks 0,1 of each chunk).

Matmuls run in float32r (fp32 truncated to 13 mantissa bits by the PE).
Inputs are pre-rounded to that grid on the host, making the on-device
truncation exact and the Gram error unbiased (~7e-5 of G's std).

Device output is the uint8 adjacency mask of the computed tiles only;
the host mirrors the triangle and expands to the int32 edge planes.
"""

import sys

for _p in ("/opt/trn_rl_repo", "/root/.axon_site/_ro/trn_rl_repo"):
    if _p not in sys.path:
        sys.path.append(_p)

import numpy as np

B, C, N = 4, 384, 4096
HALF = N // 2
KT = C // 128          # 3 contraction tiles
NCHUNK = 8             # 512-wide column chunks
CW = 512
NCORES = 2 * B
PPF_09 = 1.2815515655446004
EPS = 1e-12
MBITS = 13             # PE fp32r keeps 13 explicit mantissa bits

_compiled_nc = None


def _build_nc():
    import concourse.bacc as bacc
    import concourse.tile as tile
    import concourse.mybir as mybir

    f32 = mybir.dt.float32
    f32r = mybir.dt.float32r
    u8 = mybir.dt.uint8
    Alu = mybir.AluOpType

    nc = bacc.Bacc("TRN2", target_bir_lowering=False, debug=False)

    x_d = nc.dram_tensor("xc", [NCHUNK, 128, KT, CW], f32r, kind="ExternalInput")
    thr_d = nc.dram_tensor("thr", [128, 1], f32, kind="ExternalInput")
    m_d = nc.dram_tensor("m", [HALF, N], u8, kind="ExternalOutput")

    with tile.TileContext(nc) as tc:
        with tc.tile_pool(name="xpool", bufs=1) as xpool, \
             tc.tile_pool(name="psum", bufs=5, space="PSUM") as psum, \
             tc.tile_pool(name="mpool", bufs=4) as mpool:
            thr_t = xpool.tile([128, 1], f32, name="thr_t")
            nc.sync.dma_start(out=thr_t[:], in_=thr_d.ap())
            xc = [xpool.tile([128, KT, CW], f32r, name=f"xc{c}")
                  for c in range(NCHUNK)]
            for c in range(NCHUNK):
                nc.sync.dma_start(out=xc[c][:], in_=x_d[c])

            # wavefront over moving chunks: tiles for chunk m only need
            # chunks q <= m, so compute starts as soon as chunk 0 lands.
            for m in range(NCHUNK):
                for rb in range(2 * (m + 1)):
                    q, r = rb // 2, rb % 2
                    ps = psum.tile([128, CW], f32, name="ps")
                    for k in range(KT):
                        nc.tensor.matmul(
                            ps[:],
                            xc[q][:, k, r * 128:(r + 1) * 128],
                            xc[m][:, k, :],
                            start=(k == 0), stop=(k == KT - 1),
                        )
                    mk = mpool.tile([128, CW], u8, name="mk")
                    nc.vector.tensor_scalar(
                        mk[:], ps[:], thr_t[:], None, op0=Alu.is_gt)
                    nc.sync.dma_start(
                        out=m_d.ap()[rb * 128:(rb + 1) * 128,
                                     m * CW:(m + 1) * CW],
                        in_=mk[:])
    nc.compile()
    return nc


def get_nc():
    global _compiled_nc
    if _compiled_nc is None:
        _compiled_nc = _build_nc()
    return _compiled_nc


def _round_mant(x, bits):
    """Round fp32 array to `bits` explicit mantissa bits (fp32r grid)."""
    m, e = np.frexp(x)
    s = np.float32(1 << bits)
    m = np.round(m * s) / s
    return np.ldexp(m, e).astype(np.float32)


_PERM = ([0, 1, 2, 3], [2, 3, 0, 1])   # self-inverse block perms per h


def make_inputs(x):
    xs = np.asarray(x)[:, :, :, 0]                      # (B, C, N) fp32
    nrm = np.sqrt(np.sum(xs * xs, axis=1, keepdims=True))
    xn = xs / np.maximum(nrm, EPS)

    Nsq = float(N) * float(N)
    in_maps = []
    for b in range(B):
        xb64 = xn[b].astype(np.float64)
        s = xb64.sum(axis=1)
        M = xb64 @ xb64.T
        sum_g = float(s @ s)
        sum_g2 = float((M * M).sum())
        mean = (2.0 * sum_g - 2.0 * Nsq) / Nsq
        s2 = 4.0 * sum_g2 - 8.0 * sum_g + 4.0 * Nsq
        var = (s2 - Nsq * mean * mean) / (Nsq - 1.0)
        t_b = (mean + PPF_09 * np.sqrt(var) + 2.0) / 2.0
        thr_dev = np.full((128, 1), t_b, np.float32)

        xbr = _round_mant(xn[b].astype(np.float32), MBITS)  # (C, N)
        for h in range(2):
            xloc = xbr.reshape(C, NCHUNK, 4, 128)[:, :, _PERM[h], :]
            xloc = xloc.reshape(C, N)
            xcarr = xloc.reshape(KT, 128, NCHUNK, CW).transpose(2, 1, 0, 3)
            in_maps.append({
                "xc": np.ascontiguousarray(xcarr),
                "thr": thr_dev,
            })
    return in_maps


def assemble(results):
    out = np.empty((2, B * N * N), np.int32)
    iota = np.arange(N, dtype=np.int32)
    neg1 = np.int32(-1)
    for b in range(B):
        adjU = np.empty((N, N), np.uint8)
        for h in range(2):
            mv = results[2 * b + h]["m"]                # (HALF, N) local cols
            if h == 1:
                mv = mv.reshape(HALF, NCHUNK, 4, 128)[:, :, _PERM[1], :]
                mv = mv.reshape(HALF, N)
            mv = mv.reshape(16, 128, N)
            for rb in range(16):
                a = 4 * (rb // 2) + (rb % 2) + 2 * h
                adjU[a * 128:(a + 1) * 128] = mv[rb]
        adj = np.triu(adjU)
        adj += np.triu(adjU, 1).T
        src = b * N + iota
        out[0, b * N * N:(b + 1) * N * N] = np.where(
            adj, src[:, None], neg1).ravel()
        out[1, b * N * N:(b + 1) * N * N] = np.where(
            adj, src[None, :], neg1).ravel()
    return out


def kernel(x):
    from concourse.bass_utils import run_bass_kernel_spmd

    nc = get_nc()
    in_maps = make_inputs(x)
    res = run_bass_kernel_spmd(nc, in_maps, list(range(NCORES)))
    return assemble(res.results)


# revision 5
# speedup vs baseline: 2.9721x; 1.0559x over previous
"""v7: fp32r upper-triangle Gram; fp16 (G-thr) output; host near-band rescue.

Per batch, G = xn^T xn is symmetric: only upper-triangular [128,512] tiles
are computed (host mirrors). The two cores of a batch split every column
chunk's four 128-row blocks {0,1}/{2,3}; a per-core within-chunk block
permutation ([0,1,2,3] / [2,3,0,1]) makes the SPMD program identical.

Matmuls are single-pass float32r (fp32 truncated to ~12-13 mantissa bits
by the PE; inputs pre-rounded to that grid on host). Precision recovery:
the device outputs d = fp16(G - thr) per element (sign = adjacency,
magnitude = distance to threshold); the host recomputes the tiny
|d| < 1e-4 band exactly in float64, so the final adjacency matches the
reference to ~fp32-roundoff level.

Per-tile PSUM drain is split across the Vector and Scalar engines
(halves of the tile) so the drain (~580ns) stays below the 3-matmul
group time (~700ns) and the pipeline is PE-paced. Input chunk DMAs are
emitted interleaved with compute so output DMAs are not queued behind
the whole input stream. A short garbage-matmul warmup keeps the PE HAM
clock-gate warm before the first real matmul.
"""

import sys

for _p in ("/opt/trn_rl_repo", "/root/.axon_site/_ro/trn_rl_repo"):
    if _p not in sys.path:
        sys.path.append(_p)

import numpy as np

B, C, N = 4, 384, 4096
HALF = N // 2
KT = C // 128          # 3 contraction tiles
NCHUNK = 8             # 512-wide column chunks
CW = 512
NCORES = 2 * B
PPF_09 = 1.2815515655446004
EPS = 1e-12
MBITS = 13             # fp32r mantissa grid (pre-round on host)
MARGIN = 1e-4          # |G - thr| band recomputed exactly on host
NWARM = 14             # PE warmup matmuls (run during input DMA wait)

_compiled_nc = None


def _build_nc():
    import concourse.bacc as bacc
    import concourse.tile as tile
    import concourse.mybir as mybir

    f32 = mybir.dt.float32
    f32r = mybir.dt.float32r
    f16 = mybir.dt.float16
    Alu = mybir.AluOpType
    Act = mybir.ActivationFunctionType

    nc = bacc.Bacc("TRN2", target_bir_lowering=False, debug=False)

    x_d = nc.dram_tensor("xc", [NCHUNK, 128, KT, CW], f32r,
                         kind="ExternalInput")
    nthr_d = nc.dram_tensor("nthr", [128, 1], f32, kind="ExternalInput")
    d_d = nc.dram_tensor("d", [HALF, N], f16, kind="ExternalOutput")

    with tile.TileContext(nc) as tc:
        with tc.tile_pool(name="xpool", bufs=1) as xpool, \
             tc.tile_pool(name="psum", bufs=5, space="PSUM") as psum, \
             tc.tile_pool(name="warmp", bufs=1, space="PSUM") as warmp, \
             tc.tile_pool(name="mpool", bufs=6) as mpool:
            nthr_t = xpool.tile([128, 1], f32, name="nthr_t")
            nc.sync.dma_start(out=nthr_t[:], in_=nthr_d.ap())
            bf16 = mybir.dt.bfloat16
            dummy = xpool.tile([128, CW], bf16, name="dummy")
            nc.gpsimd.memset(dummy[:], 1.0)
            xc = [xpool.tile([128, KT, CW], f32r, name=f"xc{c}")
                  for c in range(NCHUNK)]
            # chunk 0 split per k so the first matmul starts earliest
            for k in range(KT):
                nc.sync.dma_start(out=xc[0][:, k, :], in_=x_d.ap()[0, :, k, :])
            nc.sync.dma_start(out=xc[1][:], in_=x_d[1])

            # HAM warmup: garbage matmuls into a scratch bank while the
            # input stream lands; results are never read.
            wps = warmp.tile([128, CW], f32, name="wps")
            for i in range(NWARM):
                nc.tensor.matmul(wps[:], dummy[:, 0:128], dummy[:],
                                 start=True, stop=True)

            # wavefront over moving chunks: tiles for chunk m need only
            # chunks q <= m; chunk m+2's DMA is emitted between phases so
            # output DMAs interleave with input DMAs on the queue.
            for m in range(NCHUNK):
                if m + 2 < NCHUNK:
                    nc.sync.dma_start(out=xc[m + 2][:], in_=x_d[m + 2])
                for rb in range(2 * (m + 1)):
                    q, r = rb // 2, rb % 2
                    ps = psum.tile([128, CW], f32, name="ps")
                    for k in range(KT):
                        nc.tensor.matmul(
                            ps[:],
                            xc[q][:, k, r * 128:(r + 1) * 128],
                            xc[m][:, k, :],
                            start=(k == 0), stop=(k == KT - 1),
                        )
                    dt = mpool.tile([128, CW], f16, name="dt")
                    nc.vector.tensor_scalar(
                        dt[:, 0:256], ps[:, 0:256], nthr_t[:], None,
                        op0=Alu.add)
                    nc.scalar.activation(
                        dt[:, 256:512], ps[:, 256:512], Act.Identity,
                        bias=nthr_t[:], scale=1.0)
                    nc.sync.dma_start(
                        out=d_d.ap()[rb * 128:(rb + 1) * 128,
                                     m * CW:(m + 1) * CW],
                        in_=dt[:])
    nc.compile()
    return nc


def get_nc():
    global _compiled_nc
    if _compiled_nc is None:
        _compiled_nc = _build_nc()
    return _compiled_nc


def _round_mant(x, bits):
    """Round fp32 array to `bits` explicit mantissa bits."""
    m, e = np.frexp(x)
    s = np.float32(1 << bits)
    m = np.round(m * s) / s
    return np.ldexp(m, e).astype(np.float32)


_PERM = ([0, 1, 2, 3], [2, 3, 0, 1])   # self-inverse block perms per h

_state = {}


def make_inputs(x):
    xs = np.asarray(x)[:, :, :, 0]                      # (B, C, N) fp32
    nrm = np.sqrt(np.sum(xs * xs, axis=1, keepdims=True))
    xn = xs / np.maximum(nrm, EPS)

    Nsq = float(N) * float(N)
    in_maps = []
    xn64s, thrs = [], []
    for b in range(B):
        xb64 = xn[b].astype(np.float64)
        s = xb64.sum(axis=1)
        M = xb64 @ xb64.T
        sum_g = float(s @ s)
        sum_g2 = float((M * M).sum())
        mean = (2.0 * sum_g - 2.0 * Nsq) / Nsq
        s2 = 4.0 * sum_g2 - 8.0 * sum_g + 4.0 * Nsq
        var = (s2 - Nsq * mean * mean) / (Nsq - 1.0)
        t_b = (mean + PPF_09 * np.sqrt(var) + 2.0) / 2.0
        xn64s.append(xb64)
        thrs.append(t_b)

        nthr_dev = np.full((128, 1), -t_b, np.float32)
        xbr = _round_mant(xn[b].astype(np.float32), MBITS)  # (C, N)
        for h in range(2):
            xloc = xbr.reshape(C, NCHUNK, 4, 128)[:, :, _PERM[h], :]
            xloc = xloc.reshape(C, N)
            xcarr = xloc.reshape(KT, 128, NCHUNK, CW).transpose(2, 1, 0, 3)
            in_maps.append({
                "xc": np.ascontiguousarray(xcarr),
                "nthr": nthr_dev,
            })
    _state["xn64"] = xn64s
    _state["thr"] = thrs
    return in_maps


def assemble(results):
    out = np.empty((2, B * N * N), np.int32)
    iota = np.arange(N, dtype=np.int32)
    neg1 = np.int32(-1)
    for b in range(B):
        dU = np.empty((N, N), np.float16)
        for h in range(2):
            dv = results[2 * b + h]["d"]                # (HALF, N) local cols
            if h == 1:
                dv = dv.reshape(HALF, NCHUNK, 4, 128)[:, :, _PERM[1], :]
                dv = dv.reshape(HALF, N)
            dv = dv.reshape(16, 128, N)
            for rb in range(16):
                a = 4 * (rb // 2) + (rb % 2) + 2 * h
                dU[a * 128:(a + 1) * 128] = dv[rb]
        dU32 = dU.astype(np.float32)
        adjU = (dU32 > 0).astype(np.uint8)
        nearU = np.triu(np.abs(dU32) < MARGIN)
        ii, jj = np.nonzero(nearU)
        if ii.size:
            xn64 = _state["xn64"][b]
            g = np.einsum('ci,ci->i', xn64[:, ii], xn64[:, jj])
            adjU[ii, jj] = g > _state["thr"][b]
        adj = np.triu(adjU)
        adj += np.triu(adjU, 1).T
        src = b * N + iota
        out[0, b * N * N:(b + 1) * N * N] = np.where(
            adj, src[:, None], neg1).ravel()
        out[1, b * N * N:(b + 1) * N * N] = np.where(
            adj, src[None, :], neg1).ravel()
    return out


def kernel(x):
    from concourse.bass_utils import run_bass_kernel_spmd

    nc = get_nc()
    in_maps = make_inputs(x)
    res = run_bass_kernel_spmd(nc, in_maps, list(range(NCORES)))
    return assemble(res.results)
